# revision 1
# baseline (speedup 1.0000x reference)
"""Trainium2 Bass kernel for a dense transformer block.

Block: y = x + proj(MHA(LN1(x), rel-pos-bias)) ; out = y + fc2(gelu(fc1(LN2(y))))
Shapes (hardcoded): B=4, N=2048, C=512, H=8, DH=64, HID=2048, fp32 I/O.

Sharding over 8 cores: core c -> (batch b = c//2, query-half par = c%2).
Each core receives its batch's rows rolled so its own 1024 query tokens come
first, computes K/V over all 2048 tokens (duplicated across the pair of cores
sharing a batch -- cheaper than a collective), and runs attention + MLP for its
own 1024 tokens. Weights are replicated; LayerNorm affine params are folded
into the matmul weights on the host.

On-chip layout: activations are kept transposed ([C, tok]) so that
  - scores    S^T[k,q]  = matmul(lhsT=K^T slice, rhs=Q^T slice)   (per head)
  - attnV     O^T[dh,q] = matmul(lhsT=V_aug[k,65], rhs=P^T[k,q])  (accum over k)
both run the PE at full rate. All matmul operands are bf16 (PE accumulates in
fp32); the residual stream, LN stats, scores (pre-exp) and softmax denominator
stay fp32. Bias rows and the softmax reciprocal enter matmuls as hi+lo bf16
pairs so their contribution is fp32-accurate. V is augmented with a ones
column so the attnV accumulation also produces the softmax denominator
(row 64). The relative position bias is served from host-precomputed shifted
tables so the bias tile for any (head, k-tile) is a plain free-dim slice of a
[128, 1920] SBUF block.
"""

import threading
from contextlib import ExitStack

import numpy as np

import concourse.bass as bass
import concourse.tile as tile
from concourse import bacc, mybir
from concourse.bass_utils import run_bass_kernel_spmd
from concourse.masks import make_identity

F32 = mybir.dt.float32
BF16 = mybir.dt.bfloat16

B, N, C, H = 4, 2048, 512, 8
DH = C // H          # 64
HID = 4 * C          # 2048
NQ = N // 2          # own query tokens per core (1024)
EPS = 1e-5
P = 128              # partitions
TT = N // P          # 16 token tiles (full batch)
TQ = NQ // P         # 8 token tiles (own)
CT = C // P          # 4 channel tiles
OT = HID // P        # 16 hidden tiles
BLKW = NQ + 7 * P    # 1920, bias block width


def build_program(reps: int = 1, phases: str = "abcde"):
    """Build the per-core Bass program (SPMD; all per-core differences are
    carried by input data)."""
    nc = bacc.Bacc("TRN2", target_bir_lowering=False, debug=False, num_devices=8)

    t = {}
    t["xb"] = nc.dram_tensor("xb", [N, C], F32, kind="ExternalInput").ap()
    t["wqkvT"] = nc.dram_tensor("wqkvT", [C, 3 * C], BF16,
                                kind="ExternalInput").ap()
    t["bqk"] = nc.dram_tensor("bqk", [2 * C], F32, kind="ExternalInput").ap()
    t["bv2"] = nc.dram_tensor("bv2", [2, C], BF16, kind="ExternalInput").ap()
    t["wprojT"] = nc.dram_tensor("wprojT", [C, C], BF16,
                                 kind="ExternalInput").ap()
    t["bproj2"] = nc.dram_tensor("bproj2", [2, C], BF16,
                                 kind="ExternalInput").ap()
    t["wfc1T"] = nc.dram_tensor("wfc1T", [C, HID], BF16,
                                kind="ExternalInput").ap()
    t["bfc1"] = nc.dram_tensor("bfc1", [HID], F32, kind="ExternalInput").ap()
    t["wfc2T"] = nc.dram_tensor("wfc2T", [HID, C], BF16,
                                kind="ExternalInput").ap()
    t["bfc22"] = nc.dram_tensor("bfc22", [2, C], BF16,
                                kind="ExternalInput").ap()
    t["blka"] = nc.dram_tensor("blka", [H, P, BLKW], BF16,
                               kind="ExternalInput").ap()
    t["blkb"] = nc.dram_tensor("blkb", [H, P, BLKW], BF16,
                               kind="ExternalInput").ap()
    t["oT_d"] = nc.dram_tensor("oT_d", [C, NQ], BF16).ap()
    t["dbg"] = nc.dram_tensor("dbg", [C, N], BF16).ap()
    t["phases"] = phases
    t["out"] = nc.dram_tensor("out", [NQ, C], F32, kind="ExternalOutput").ap()

    with tile.TileContext(nc) as tc:
        if reps == 1:
            _build_body(nc, tc, t)
        else:
            with tc.For_i(0, reps, 1):
                _build_body(nc, tc, t)
    nc.compile()
    return nc


def _build_body(nc, tc, t):
    Act = mybir.ActivationFunctionType
    Alu = mybir.AluOpType

    xb, out, oT_d = t["xb"], t["out"], t["oT_d"]

    with ExitStack() as ctx:
        singles = ctx.enter_context(tc.tile_pool(name="singles", bufs=1))
        ident = singles.tile([P, P], F32)
        make_identity(nc, ident)
        eps_t = singles.tile([P, 1], F32)
        nc.gpsimd.memset(eps_t, EPS)
        ones2 = singles.tile([2, P], BF16)
        nc.gpsimd.memset(ones2, 1.0)

        x_own = [None] * TQ
        kT = [None] * CT
        qT = [None] * CT
        va = [None] * TT
        oT = [None] * CT

        ad = ctx.enter_context(ExitStack())   # spans phases A..D
        xq_pool = ad.enter_context(tc.tile_pool(name="xq", bufs=TQ))
        ac_scope = ad.enter_context(ExitStack())  # spans phases A..C
        kT_pool = ac_scope.enter_context(tc.tile_pool(name="kT", bufs=CT))
        qT_pool = ac_scope.enter_context(tc.tile_pool(name="qT", bufs=CT))
        va_pool = ac_scope.enter_context(tc.tile_pool(name="va", bufs=TT))

        # ------------------------------------------------------------------
        # Phases A+B: LN1 (transposed) and QKV projections
        # ------------------------------------------------------------------
        with ExitStack() as ab:
            z1t_pool = ab.enter_context(tc.tile_pool(name="z1t", bufs=CT))
            xload_pool = ab.enter_context(tc.tile_pool(name="xload", bufs=3))
            stat_pool = ab.enter_context(tc.tile_pool(name="stat", bufs=4))
            wq_pool = ab.enter_context(tc.tile_pool(name="wq", bufs=CT))
            bias_pool = ab.enter_context(tc.tile_pool(name="qkvb", bufs=1))
            tpsum = ab.enter_context(
                tc.tile_pool(name="tpsum", bufs=4, space="PSUM"))
            bpsum = ab.enter_context(
                tc.tile_pool(name="bpsum", bufs=4, space="PSUM"))

            z1t = [z1t_pool.tile([P, N], BF16, tag="z1t", name=f"z1t{i}")
                   for i in range(CT)]

            # --- A: LayerNorm1 (stats in natural layout, PE transpose) ---
            for tt in range(TT):
                if tt < TQ:
                    x_t = xq_pool.tile([P, C], F32, tag="xq")
                    x_own[tt] = x_t
                else:
                    x_t = xload_pool.tile([P, C], F32, tag="xload")
                nc.sync.dma_start(out=x_t, in_=xb[tt * P:(tt + 1) * P, :])
                st = stat_pool.tile([P, 6], F32, tag="st")
                mv = stat_pool.tile([P, 2], F32, tag="mv")
                nc.vector.bn_stats(out=st, in_=x_t)
                nc.vector.bn_aggr(out=mv, in_=st)
                rs = stat_pool.tile([P, 1], F32, tag="rs")
                nc.scalar.activation(out=rs, in_=mv[:, 1:2], func=Act.Sqrt,
                                     bias=eps_t, scale=1.0)
                nc.vector.reciprocal(out=rs, in_=rs)
                z_t = xload_pool.tile([P, C], F32, tag="zt")
                nc.vector.tensor_scalar(out=z_t, in0=x_t, scalar1=mv[:, 0:1],
                                        scalar2=rs, op0=Alu.subtract,
                                        op1=Alu.mult)
                for ct in range(CT):
                    pt = tpsum.tile([P, P], F32, tag="tr")
                    nc.tensor.transpose(pt, z_t[:, ct * P:(ct + 1) * P], ident)
                    nc.any.tensor_copy(
                        out=z1t[ct][:, tt * P:(tt + 1) * P], in_=pt)

            # --- B: QKV ---
            wsb = []
            for ct in range(CT):
                w_t = wq_pool.tile([P, 3 * C], BF16, tag="wq")
                nc.sync.dma_start(
                    out=w_t, in_=t["wqkvT"][ct * P:(ct + 1) * P, :])
                wsb.append(w_t)
            # per-o-tile fp32 bias columns for Q (o 0..3) and K (o 4..7)
            bcols = []
            for ot in range(8):
                bt = bias_pool.tile([P, 1], F32, tag="bcol", bufs=8)
                nc.sync.dma_start(
                    out=bt,
                    in_=t["bqk"][ot * P:(ot + 1) * P].rearrange(
                        "(p one) -> p one", one=1))
                bcols.append(bt)
            bvrow = bias_pool.tile([2, C], BF16, tag="bvrow")
            nc.sync.dma_start(out=bvrow, in_=t["bv2"])

            # V natural [tok, 512] + ones column per head -> [P, H, 65]
            for tt in range(TT):
                v_t = va_pool.tile([P, H * (DH + 1)], BF16, tag="va")
                va[tt] = v_t
                nc.gpsimd.memset(v_t, 1.0)
                ps = bpsum.tile([P, 512], F32, tag="mm")
                for ct in range(CT):
                    nc.tensor.matmul(
                        ps,
                        lhsT=z1t[ct][:, tt * P:(tt + 1) * P],
                        rhs=wsb[ct][:, 2 * C:3 * C],
                        start=(ct == 0), stop=False)
                nc.tensor.matmul(ps, lhsT=ones2, rhs=bvrow,
                                 start=False, stop=True)
                nc.vector.tensor_copy(
                    out=v_t.rearrange("p (h w) -> p h w", w=DH + 1)[:, :, 0:DH],
                    in_=ps.rearrange("p (h w) -> p h w", w=DH))

            # K^T: heads along partitions (o-tiles 4..7 of qkv), all N tokens
            for ot in range(CT):
                k_t = kT_pool.tile([P, N], BF16, tag="kT")
                kT[ot] = k_t
                for tch in range(N // 512):
                    ps = bpsum.tile([P, 512], F32, tag="mm")
                    for ct in range(CT):
                        nc.tensor.matmul(
                            ps,
                            lhsT=wsb[ct][:, C + ot * P:C + (ot + 1) * P],
                            rhs=z1t[ct][:, tch * 512:(tch + 1) * 512],
                            start=(ct == 0), stop=(ct == CT - 1))
                    nc.vector.tensor_scalar_add(
                        out=k_t[:, tch * 512:(tch + 1) * 512], in0=ps,
                        scalar1=bcols[4 + ot])
            # Q^T: o-tiles 0..3, own tokens only (first NQ columns of z1t)
            for ot in range(CT):
                q_t = qT_pool.tile([P, NQ], BF16, tag="qT")
                qT[ot] = q_t
                for tch in range(NQ // 512):
                    ps = bpsum.tile([P, 512], F32, tag="mm")
                    for ct in range(CT):
                        nc.tensor.matmul(
                            ps,
                            lhsT=wsb[ct][:, ot * P:(ot + 1) * P],
                            rhs=z1t[ct][:, tch * 512:(tch + 1) * 512],
                            start=(ct == 0), stop=(ct == CT - 1))
                    nc.vector.tensor_scalar_add(
                        out=q_t[:, tch * 512:(tch + 1) * 512], in0=ps,
                        scalar1=bcols[ot])
        if "c" not in t["phases"]:
            for ot in range(CT):
                nc.sync.dma_start(out=t["dbg"][ot * P:(ot + 1) * P, :],
                                  in_=kT[ot])
                nc.sync.dma_start(out=t["dbg"][ot * P:(ot + 1) * P, 0:NQ],
                                  in_=qT[ot])
            for tt in range(TT):
                nc.sync.dma_start(
                    out=t["dbg"][0:P, tt * P:(tt + 1) * P],
                    in_=va[tt][:, 0:P])
            for tq in range(TQ):
                o_t = xq_pool.tile([P, C], F32, tag="dumout", bufs=2)
                nc.vector.tensor_copy(out=o_t, in_=x_own[tq])
                nc.sync.dma_start(out=out[tq * P:(tq + 1) * P, :], in_=o_t)
            return

        # ------------------------------------------------------------------
        # Phase C: attention, head-pairs, flash-style over k tiles
        # ------------------------------------------------------------------
        with ExitStack() as cx:
            ostg_pool = cx.enter_context(tc.tile_pool(name="ostg", bufs=4))
            blk_pool = cx.enter_context(tc.tile_pool(name="blk", bufs=8))
            t_pool = cx.enter_context(tc.tile_pool(name="texp", bufs=8))
            d_pool = cx.enter_context(tc.tile_pool(name="den", bufs=2))
            spsum = cx.enter_context(
                tc.tile_pool(name="spsum", bufs=4, space="PSUM"))
            acpsum = cx.enter_context(
                tc.tile_pool(name="acpsum", bufs=2, space="PSUM"))

            for hp in range(H // 2):
                h0, h1 = 2 * hp, 2 * hp + 1
                blks = {}
                for (hh, loc) in ((h0, 0), (h1, 1)):
                    for half, src in ((0, t["blka"]), (1, t["blkb"])):
                        bb = blk_pool.tile([P, BLKW], BF16, tag="blk",
                                           name=f"blk{hp}_{hh}_{half}")
                        nc.sync.dma_start(out=bb, in_=src[hh])
                        blks[(loc, half)] = bb
                ac = [acpsum.tile([DH + 1, NQ], F32, tag="ac",
                                  name=f"ac{hp}_{i}")
                      for i in range(2)]
                # software-pipelined by one k-tile: attnV matmuls for kt
                # are emitted after kt+1's scores so the in-order PE stream
                # never blocks on the DVE->ACT softmax chain.
                pend = []
                for kt in range(TT):
                    half = 0 if kt < 8 else 1
                    off = (7 - kt % 8) * P
                    cur = []
                    for loc, hh in ((0, h0), (1, h1)):
                        sf = t_pool.tile([P, NQ], F32, tag="sf", bufs=6,
                                         name=f"sf{hp}_{kt}_{loc}")
                        texp = t_pool.tile([P, NQ], BF16, tag="texp", bufs=8,
                                           name=f"texp{hp}_{kt}_{loc}")
                        for qc in range(NQ // 512):
                            sl = slice(qc * 512, (qc + 1) * 512)
                            sp = spsum.tile([P, 512], F32, tag="sc",
                                            name=f"sc{hp}_{kt}_{loc}_{qc}")
                            nc.tensor.matmul(
                                sp,
                                lhsT=kT[hp][loc * DH:(loc + 1) * DH,
                                            kt * P:(kt + 1) * P],
                                rhs=qT[hp][loc * DH:(loc + 1) * DH, sl],
                                start=True, stop=True)
                            # DVE: evacuate PSUM with +8*bias fused
                            nc.vector.tensor_tensor(
                                out=sf[:, sl], in0=sp,
                                in1=blks[(loc, half)][:, off + qc * 512:
                                                      off + (qc + 1) * 512],
                                op=Alu.add)
                        # ACT: one exp over the full row block (SBUF source)
                        nc.scalar.activation(out=texp, in_=sf, func=Act.Exp,
                                             scale=float(DH) ** -0.5)
                        cur.append((loc, hh, texp))
                    for loc, hh, texp in pend:
                        for qc in range(NQ // 512):
                            sl = slice(qc * 512, (qc + 1) * 512)
                            nc.tensor.matmul(
                                ac[loc][:, sl],
                                lhsT=va[kt - 1][:, hh * (DH + 1):
                                                (hh + 1) * (DH + 1)],
                                rhs=texp[:, sl],
                                start=(kt - 1 == 0), stop=False)
                    pend = cur
                for loc, hh, texp in pend:
                    for qc in range(NQ // 512):
                        sl = slice(qc * 512, (qc + 1) * 512)
                        nc.tensor.matmul(
                            ac[loc][:, sl],
                            lhsT=va[TT - 1][:, hh * (DH + 1):
                                            (hh + 1) * (DH + 1)],
                            rhs=texp[:, sl],
                            start=False, stop=True)
                # normalize: O^T = num * (1/den), 1/den as bf16 hi+lo pair
                for loc in range(2):
                    rden = d_pool.tile([1, NQ], F32, tag="rden")
                    nc.vector.reciprocal(out=rden, in_=ac[loc][DH:DH + 1, :])
                    rden_hi = d_pool.tile([1, NQ], BF16, tag="rdenh")
                    rden_lo = d_pool.tile([1, NQ], BF16, tag="rdenl")
                    nc.vector.tensor_copy(out=rden_hi, in_=rden)
                    nc.vector.tensor_tensor(out=rden_lo, in0=rden,
                                            in1=rden_hi, op=Alu.subtract)
                    ost = ostg_pool.tile([DH, NQ], BF16, tag="ostg",
                                         name=f"ostg{hp}_{loc}")
                    pbs = d_pool.tile([DH, NQ], F32, tag="pbs")
                    for qc in range(NQ // 512):
                        sl = slice(qc * 512, (qc + 1) * 512)
                        pb = spsum.tile([DH, 512], F32, tag="sc",
                                        name=f"pb{hp}_{loc}_{qc}")
                        nc.tensor.matmul(pb, lhsT=ones2[0:1, 0:DH],
                                         rhs=rden_hi[:, sl],
                                         start=True, stop=False)
                        nc.tensor.matmul(pb, lhsT=ones2[0:1, 0:DH],
                                         rhs=rden_lo[:, sl],
                                         start=False, stop=True)
                        nc.scalar.copy(out=pbs[:, sl], in_=pb)
                        nc.vector.tensor_tensor(
                            out=ost[:, sl],
                            in0=ac[loc][0:DH, sl], in1=pbs[:, sl],
                            op=Alu.mult)
                    hh = 2 * hp + loc
                    nc.sync.dma_start(
                        out=oT_d[hh * DH:(hh + 1) * DH, :], in_=ost)
        ac_scope.close()  # free kT/qT/va before phases D/E need SBUF

        if "d" not in t["phases"]:
            for tq in range(TQ):
                o_t = xq_pool.tile([P, C], F32, tag="dumout", bufs=2)
                nc.vector.tensor_copy(out=o_t, in_=x_own[tq])
                nc.sync.dma_start(out=out[tq * P:(tq + 1) * P, :], in_=o_t)
            return

        # ------------------------------------------------------------------
        # Phase D: proj + residual + LN2 (transposed)
        # ------------------------------------------------------------------
        x2_pool = ctx.enter_context(tc.tile_pool(name="x2", bufs=TQ))
        z2t_pool = ctx.enter_context(tc.tile_pool(name="z2t", bufs=CT))
        z2t = [z2t_pool.tile([P, NQ], BF16, tag="z2t", name=f"z2t{i}")
               for i in range(CT)]
        x2 = [None] * TQ
        with ExitStack() as dx:
            wp_pool = dx.enter_context(tc.tile_pool(name="wp", bufs=CT))
            brow_pool = dx.enter_context(tc.tile_pool(name="brow", bufs=1))
            stat2_pool = dx.enter_context(tc.tile_pool(name="stat2", bufs=4))
            ztmp_pool = dx.enter_context(tc.tile_pool(name="ztmp", bufs=3))
            oTld_pool = dx.enter_context(tc.tile_pool(name="oTld", bufs=CT))
            dpsum = dx.enter_context(
                tc.tile_pool(name="dpsum", bufs=2, space="PSUM"))
            tpsum2 = dx.enter_context(
                tc.tile_pool(name="tpsum2", bufs=4, space="PSUM"))

            for ct in range(CT):
                o_ld = oTld_pool.tile([P, NQ], BF16, tag="oTld",
                                      name=f"oTld{ct}")
                nc.sync.dma_start(out=o_ld, in_=oT_d[ct * P:(ct + 1) * P, :])
                oT[ct] = o_ld
            wpsb = []
            for ct in range(CT):
                w_t = wp_pool.tile([P, C], BF16, tag="wp")
                nc.sync.dma_start(out=w_t,
                                  in_=t["wprojT"][ct * P:(ct + 1) * P, :])
                wpsb.append(w_t)
            bprow = brow_pool.tile([2, C], BF16, tag="bprow")
            nc.sync.dma_start(out=bprow, in_=t["bproj2"])

            for tq in range(TQ):
                ps = dpsum.tile([P, C], F32, tag="mm")
                for ct in range(CT):
                    nc.tensor.matmul(
                        ps, lhsT=oT[ct][:, tq * P:(tq + 1) * P],
                        rhs=wpsb[ct], start=(ct == 0), stop=False)
                nc.tensor.matmul(ps, lhsT=ones2, rhs=bprow,
                                 start=False, stop=True)
                x2_t = x2_pool.tile([P, C], F32, tag="x2")
                x2[tq] = x2_t
                nc.vector.tensor_add(out=x2_t, in0=ps, in1=x_own[tq])
                # LN2
                st = stat2_pool.tile([P, 6], F32, tag="st2")
                mv = stat2_pool.tile([P, 2], F32, tag="mv2")
                nc.vector.bn_stats(out=st, in_=x2_t)
                nc.vector.bn_aggr(out=mv, in_=st)
                rs = stat2_pool.tile([P, 1], F32, tag="rs2")
                nc.scalar.activation(out=rs, in_=mv[:, 1:2], func=Act.Sqrt,
                                     bias=eps_t, scale=1.0)
                nc.vector.reciprocal(out=rs, in_=rs)
                z_t = ztmp_pool.tile([P, C], F32, tag="z2tmp")
                nc.vector.tensor_scalar(out=z_t, in0=x2_t, scalar1=mv[:, 0:1],
                                        scalar2=rs, op0=Alu.subtract,
                                        op1=Alu.mult)
                for ct in range(CT):
                    pt = tpsum2.tile([P, P], F32, tag="tr2")
                    nc.tensor.transpose(pt, z_t[:, ct * P:(ct + 1) * P], ident)
                    nc.any.tensor_copy(
                        out=z2t[ct][:, tq * P:(tq + 1) * P], in_=pt)

        # ------------------------------------------------------------------
        # Phase E: MLP
        # ------------------------------------------------------------------
        with ExitStack() as ex:
            w1_pool = ex.enter_context(tc.tile_pool(name="w1", bufs=CT))
            g_pool = ex.enter_context(tc.tile_pool(name="g", bufs=OT))
            w2_pool = ex.enter_context(tc.tile_pool(name="w2", bufs=OT))
            b1_pool = ex.enter_context(tc.tile_pool(name="b1", bufs=1))
            out_pool = ex.enter_context(tc.tile_pool(name="outp", bufs=2))
            epsum = ex.enter_context(
                tc.tile_pool(name="epsum", bufs=4, space="PSUM"))

            w1sb = []
            for ct in range(CT):
                w_t = w1_pool.tile([P, HID], BF16, tag="w1")
                nc.sync.dma_start(out=w_t,
                                  in_=t["wfc1T"][ct * P:(ct + 1) * P, :])
                w1sb.append(w_t)
            b1cols = []
            for ot in range(OT):
                bt = b1_pool.tile([P, 1], F32, tag="b1c", bufs=OT)
                nc.sync.dma_start(
                    out=bt, in_=t["bfc1"][ot * P:(ot + 1) * P].rearrange(
                        "(p one) -> p one", one=1))
                b1cols.append(bt)
            b2row = b1_pool.tile([2, C], BF16, tag="b2row")
            nc.sync.dma_start(out=b2row, in_=t["bfc22"])

            gT = []
            for ot in range(OT):
                g_t = g_pool.tile([P, NQ], BF16, tag="g")
                gT.append(g_t)
                for qc in range(NQ // 512):
                    ps = epsum.tile([P, 512], F32, tag="mm1")
                    for ct in range(CT):
                        nc.tensor.matmul(
                            ps,
                            lhsT=w1sb[ct][:, ot * P:(ot + 1) * P],
                            rhs=z2t[ct][:, qc * 512:(qc + 1) * 512],
                            start=(ct == 0), stop=(ct == CT - 1))
                    u_t = g_pool.tile([P, 512], F32, tag="u", bufs=3,
                                      name=f"u{ot}_{qc}")
                    nc.vector.tensor_scalar_add(out=u_t, in0=ps,
                                                scalar1=b1cols[ot])
                    nc.scalar.activation(out=g_t[:, qc * 512:(qc + 1) * 512],
                                         in_=u_t, func=Act.Gelu)
            w2sb = []
            for ot in range(OT):
                w_t = w2_pool.tile([P, C], BF16, tag="w2")
                nc.sync.dma_start(out=w_t,
                                  in_=t["wfc2T"][ot * P:(ot + 1) * P, :])
                w2sb.append(w_t)
            for tq in range(TQ):
                ps = epsum.tile([P, C], F32, tag="mm1")
                for ot in range(OT):
                    nc.tensor.matmul(
                        ps, lhsT=gT[ot][:, tq * P:(tq + 1) * P],
                        rhs=w2sb[ot], start=(ot == 0), stop=False)
                nc.tensor.matmul(ps, lhsT=ones2, rhs=b2row,
                                 start=False, stop=True)
                o_t = out_pool.tile([P, C], F32, tag="out")
                nc.vector.tensor_add(out=o_t, in0=ps, in1=x2[tq])
                nc.sync.dma_start(out=out[tq * P:(tq + 1) * P, :], in_=o_t)


# ---------------------------------------------------------------------------
# Host side
# ---------------------------------------------------------------------------

def _hi_lo(b):
    """Split fp32 row vector into bf16 hi + lo rows (hi + lo ~= b in fp32)."""
    import ml_dtypes
    b = np.asarray(b, np.float32)
    hi = b.astype(ml_dtypes.bfloat16)
    lo = (b - hi.astype(np.float32)).astype(ml_dtypes.bfloat16)
    return np.ascontiguousarray(np.stack([hi, lo], axis=0))


def prepare_inputs(x, qkv_w, proj_w, proj_b, rpb_table, n1_w, n1_b, n2_w, n2_b,
                   fc1_w, fc1_b, fc2_w, fc2_b):
    """Fold LN affines into weights, pre-transpose, build shifted bias blocks,
    and produce the 8 per-core input maps."""
    import ml_dtypes
    f = np.float32
    bf = ml_dtypes.bfloat16
    x = np.asarray(x, f)
    qkv_w = np.asarray(qkv_w, f)
    proj_w = np.asarray(proj_w, f)
    rpb = np.asarray(rpb_table, f)
    fc1_w = np.asarray(fc1_w, f)
    fc2_w = np.asarray(fc2_w, f)
    n1_w = np.asarray(n1_w, f); n1_b = np.asarray(n1_b, f)
    n2_w = np.asarray(n2_w, f); n2_b = np.asarray(n2_b, f)

    wqkvT = np.ascontiguousarray((qkv_w * n1_w[None, :]).T.astype(bf))
    bqkv = (qkv_w @ n1_b).astype(f)
    wprojT = np.ascontiguousarray(proj_w.T.astype(bf))
    wfc1T = np.ascontiguousarray((fc1_w * n2_w[None, :]).T.astype(bf))
    bfc1x = (np.asarray(fc1_b, f) + fc1_w @ n2_b).astype(f)
    wfc2T = np.ascontiguousarray(fc2_w.T.astype(bf))

    # bias blocks: value at (k-tile kt, partition p, own-query j) must be
    # rpb[k_glob - q_glob + N-1, h]; with own-first rolled rows and the view
    # i = j + (7 - kt%8)*128,
    #   half A (kt 0..7):  idx = 2943 + p - i
    #   half B (kt 8..15): idx = 3967 - 2048*parity + p - i
    # blocks hold 8*bias in bf16 (added to raw scores pre-softmax; the 1/8
    # scale is applied inside the exp activation)
    ii = np.arange(BLKW)[None, :]
    pp = np.arange(P)[:, None]
    idx_a = 2943 + pp - ii
    scale8 = float(DH) ** 0.5
    blka_np = np.ascontiguousarray(
        (rpb[idx_a, :] * scale8).transpose(2, 0, 1).astype(bf))
    blkb_np = []
    for par in range(2):
        idx_b = 3967 - 2048 * par + pp - ii
        blkb_np.append(np.ascontiguousarray(
            (rpb[idx_b, :] * scale8).transpose(2, 0, 1).astype(bf)))

    shared = dict(
        wqkvT=wqkvT,
        bqk=np.ascontiguousarray(bqkv[:2 * C]),
        bv2=_hi_lo(bqkv[2 * C:]),
        wprojT=wprojT,
        bproj2=_hi_lo(proj_b),
        wfc1T=wfc1T, bfc1=bfc1x, wfc2T=wfc2T,
        bfc22=_hi_lo(fc2_b),
        blka=blka_np,
    )
    in_maps = []
    for core in range(8):
        b, par = core // 2, core % 2
        xb_c = np.ascontiguousarray(np.roll(x[b], -par * NQ, axis=0))
        m = dict(shared)
        m["xb"] = xb_c
        m["blkb"] = blkb_np[par]
        in_maps.append(m)
    return in_maps


def assemble_output(results):
    out = np.empty((B, N, C), np.float32)
    for core in range(8):
        b, par = core // 2, core % 2
        out[b, par * NQ:(par + 1) * NQ, :] = results[core]["out"]
    return out


_cache = threading.local()


def _get_program():
    nc = getattr(_cache, "nc", None)
    if nc is None:
        nc = build_program(reps=1)
        _cache.nc = nc
    return nc


def kernel(**inputs) -> np.ndarray:
    in_maps = prepare_inputs(**inputs)
    nc = _get_program()
    res = run_bass_kernel_spmd(nc, in_maps, list(range(8)))
    return assemble_output(res.results)


if __name__ == "__main__":
    rng = np.random.default_rng(0)
    ins = {
        "x": rng.standard_normal((B, N, C)).astype(np.float32),
        "qkv_w": (rng.standard_normal((3 * C, C)) * 0.02).astype(np.float32),
        "proj_w": (rng.standard_normal((C, C)) * 0.02).astype(np.float32),
        "proj_b": np.zeros(C, np.float32),
        "rpb_table": (rng.standard_normal((2 * N - 1, H)) * 0.02).astype(np.float32),
        "n1_w": np.ones(C, np.float32), "n1_b": np.zeros(C, np.float32),
        "n2_w": np.ones(C, np.float32), "n2_b": np.zeros(C, np.float32),
        "fc1_w": (rng.standard_normal((HID, C)) * 0.02).astype(np.float32),
        "fc1_b": rng.standard_normal(HID).astype(np.float32),
        "fc2_w": (rng.standard_normal((C, HID)) * 0.02).astype(np.float32),
        "fc2_b": rng.standard_normal(C).astype(np.float32),
    }
    out = kernel(**ins)
    print("out", out.shape, out.dtype, float(np.abs(out).mean()))

